# revision 7
# baseline (speedup 1.0000x reference)
"""Multi-head attention (with the reference's double-split_proj quirk) on 8
Trainium2 NeuronCores via Bass/Tile.

Sharding: core c handles batch b = c//4 and heads {4g..4g+3} where g = c%4
(data parallel on B, tensor parallel on heads). The double application of
_split_proj in the reference means the Q used for head i at attention row
j = (S/16)*h' + jj is q_proj[b, 16*jj + i, 64*h' + d]; per head that is a
gather of rows s ≡ i (mod 16) of q_proj over all 16 channel groups, so each
core only needs the 1/4 of query rows with s mod 16 in its head range —
sliced on the host — and the device-side "shuffle" reduces to reading
column blocks of the projected tile. Everything runs transposed (channels
on partitions, positions on free): scores^T = K^T-chunks @ Q^T, softmax
statistics come from an appended ones-column on V (row 64 of the P@V
output is the rowsum Z), masking is a post-exp multiply by (1-mask)^T in
bf16, and softmax normalization uses a rank-1 PE outer-product broadcast
of 1/Z (no DRAM round trip).

No collectives: instead of AllGather-ing per-core features for a
tensor-parallel output projection, each core multiplies its 4 heads'
features against its 256 rows of Wo^T, producing a full-width PARTIAL
output [D, S]; the host sums the 4 partials per batch and adds bo. That
removes the AllGather (and its inter-core sync) from the per-dispatch
critical path at the cost of an 8 MB (vs 2 MB) result DMA, which is
device-local and cheap.

All device inputs arrive in ONE packed bf16 tensor per core ("pack"):
qT/kT/vT/mask/weights as bf16, the f32 bias rows appended as raw bytes
and bitcast back to f32 on the device side. One input + one output keeps
the per-dispatch host/tunnel argument-marshaling cost at its floor
(measured ~36us per extra arg per dispatch on the axon PJRT path).

Streamed matmul inputs travel as bf16 to halve DMA; the projected
Q^T/K^T stay float32r (full-rate fp32) so the scores matmul keeps
fp32-class precision, and P@V runs bf16 because the mask multiply must
run on VectorE in bf16 2x mode. All accumulation is fp32 in PSUM.
"""

import sys

for _p in ("/opt/trn_rl_repo",):
    if _p not in sys.path:
        sys.path.append(_p)

import numpy as np
import ml_dtypes

import concourse.bass as bass
import concourse.bacc as bacc
import concourse.mybir as mybir
import concourse.tile as tile
from concourse.bass_utils import run_bass_kernel_spmd

B = 2
D = 1024
H = 16
DH = 64
NCORES = 8
S_FULL = 2048

f32 = mybir.dt.float32
f32r = mybir.dt.float32r
bf16 = mybir.dt.bfloat16

_MODULES = {}


def _layout(S):
    """Element offsets (bf16) of each logical tensor inside the pack."""
    SQ = S // 4
    off = {}
    o = 0

    def add(name, n):
        nonlocal o
        off[name] = o
        o += n

    add("qT", D * SQ)        # [D, SQ] bf16
    add("kT", D * S)         # [D, S] bf16
    add("vT", D * S)         # [D, S] bf16
    add("mb", S * S)         # [S, S] bf16  (1 - mask)^T
    add("wq", D * D)         # [D, D] bf16  Wq^T
    add("wk", D * 256)       # [D, 256] bf16
    add("wv", D * 256)       # [D, 256] bf16
    add("wo", 256 * D)       # [256, D] bf16  Wo^T rows for local feats
    add("bq", 2 * D)         # [128, D//128] f32 (2 bf16 slots per f32)
    add("bk", 2 * 256)       # [128, 2] f32
    add("bvrow", 2 * 256)    # [1, 256] f32
    add("ones", 2 * 128)     # [1, 128] f32 (all ones)
    return off, o


def build_module(S):
    """Build + compile the per-core Bass module (same program on all cores)."""
    JJ = S // 16          # jj count per head (rows s = 16*jj + i)
    SQ = 4 * JJ           # host-gathered query rows per core (4 heads)
    KC = S // 128         # number of 128-wide key chunks
    NQ = min(512, S)      # matmul free-dim chunk for q/positions
    NB = S // NQ          # chunks of S at NQ
    SCH = S // 128        # s-chunks for the V projection

    OFF, NELEM = _layout(S)

    nc = bacc.Bacc("TRN2", target_bir_lowering=False, debug=False,
                   num_devices=NCORES)

    pack_d = nc.dram_tensor("pack", [NELEM], bf16, kind="ExternalInput")
    out_d = nc.dram_tensor("ofinal", [D, S], f32, kind="ExternalOutput")

    whole = pack_d[:]

    def V(name, rows, cols):
        return bass.AP(whole.tensor, whole.offset + OFF[name],
                       [[cols, rows], [1, cols]])

    def Vf(name, rows, cols, dt_=f32):
        ap = bass.AP(whole.tensor, whole.offset + OFF[name],
                     [[2 * cols, rows], [1, 2 * cols]])
        return ap.bitcast(dt_)

    qT_d = V("qT", D, SQ)
    kT_d = V("kT", D, S)
    vT_d = V("vT", D, S)
    maskb_d = V("mb", S, S)
    wqT_d = V("wq", D, D)
    wkT_d = V("wk", D, 256)
    wvT_d = V("wv", D, 256)
    woT_d = V("wo", 256, D)
    bq_d = Vf("bq", 128, D // 128)
    bk_d = Vf("bk", 128, 2)
    bvrow_d = Vf("bvrow", 1, 256, f32r)
    ones_d = Vf("ones", 1, 128, f32r)

    Exp = mybir.ActivationFunctionType.Exp
    Ident = mybir.ActivationFunctionType.Identity

    with tile.TileContext(nc) as tc:
        with (
            tc.tile_pool(name="persist", bufs=1) as pp,
            tc.tile_pool(name="stream", bufs=1) as sp,
            tc.tile_pool(name="psB", bufs=1, space="PSUM") as psB,
            tc.tile_pool(name="psC", bufs=1, space="PSUM") as psC,
        ):
            SCW = S // 2 if S >= 1024 else S
            SCB = 2 if S >= 1024 else 1

            def bigB(name):
                return psB.tile([128, SCW], f32, tag="sc", bufs=SCB, name=name)

            def bigC(name):
                return psC.tile([128, S], f32, tag="pv", name=name)

            # ---------------- constants + resident weights ----------------
            bq_sb = pp.tile([128, D // 128], f32, tag="bq")
            nc.sync.dma_start(bq_sb[:], bq_d)
            bk_sb = pp.tile([128, 2], f32, tag="bk")
            nc.sync.dma_start(bk_sb[:], bk_d)
            bvrow_sb = pp.tile([1, 256], f32r, tag="bvrow")
            nc.sync.dma_start(bvrow_sb[:], bvrow_d)
            onesrow_sb = pp.tile([1, 128], f32r, tag="onesrow")
            nc.sync.dma_start(onesrow_sb[:], ones_d)

            # qT next: the Q projection is the critical path at kernel start
            qts = []
            for ci in range(8):
                t = sp.tile([128, SQ], bf16, tag=f"qts{ci}", name=f"qts{ci}")
                eng = nc.sync if ci % 2 == 0 else nc.gpsimd
                eng.dma_start(t[:], qT_d[128 * ci:128 * (ci + 1), :])
                qts.append(t)

            # per-head Q^T and K^T packed in pairs: head hi lives on partitions
            # 64*(hi%2) .. +64 of pair tile hi//2
            QTp = [pp.tile([128, S], f32r, tag=f"QTp{h}", name=f"QTp{h}") for h in range(2)]
            KTp = [pp.tile([128, S], f32r, tag=f"KTp{h}", name=f"KTp{h}") for h in range(2)]

            def QTs(hi):
                return QTp[hi // 2][64 * (hi % 2):64 * (hi % 2) + 64, :]

            def KTs(hi):
                return KTp[hi // 2][64 * (hi % 2):64 * (hi % 2) + 64, :]

            # V+bias, augmented with a ones column per head: cols 65*hi+d (d<64), ones at 65*hi+64
            VA = [pp.tile([128, 260], bf16, tag=f"VA{sc}", name=f"VA{sc}") for sc in range(SCH)]
            for sc in range(SCH):
                nc.vector.memset(VA[sc].rearrange("p (h x) -> p h x", h=4)[:, :, 64:65], 1.0)

            # local attention features (4 heads = 256 channels), SBUF-resident
            featsb = [pp.tile([128, S], bf16, tag=f"featsb{x}", name=f"featsb{x}")
                      for x in range(2)]

            # ---------------- projections ----------------
            # Q projection: q_projT chunk p covers channel groups h' = 2p, 2p+1.
            # Wq streams in (128, 512) loads covering 4 p-chunks; the four
            # per-chunk accumulation groups interleave in one B-slot tile.
            NPQ = min(8, max(1, SCW // SQ))   # p-chunks per B psum tile
            qps = {}
            for g in range((8 + NPQ - 1) // NPQ):
                qps[g] = bigB(f"qpsB{g}")
            # interleaved per-slice accumulation groups need bank-aligned
            # slices (SQ == 512); otherwise load Wq per chunk
            NWQ = 4 if SQ >= 512 else 1       # p-chunks per Wq load
            for pg in range(8 // NWQ):
                slices = [qps[(NWQ * pg + j) // NPQ]
                          [:, SQ * ((NWQ * pg + j) % NPQ):SQ * ((NWQ * pg + j) % NPQ + 1)]
                          for j in range(NWQ)]
                for ci in range(8):
                    wq_t = sp.tile([128, 128 * NWQ], bf16, tag="wq", bufs=3, name=f"wq{pg}_{ci}")
                    eng = nc.sync if ci % 2 == 0 else nc.gpsimd
                    eng.dma_start(wq_t[:], wqT_d[128 * ci:128 * (ci + 1), 128 * NWQ * pg:128 * NWQ * (pg + 1)])
                    for j in range(NWQ):
                        nc.tensor.matmul(slices[j], wq_t[:, 128 * j:128 * (j + 1)], qts[ci][:],
                                         start=(ci == 0), stop=(ci == 7))
                for j in range(NWQ):
                    p = NWQ * pg + j
                    ps = slices[j]
                    for hi in range(4):
                        for half in range(2):
                            h2 = 2 * p + half
                            nc.vector.tensor_scalar_add(
                                QTs(hi)[:, JJ * h2:JJ * (h2 + 1)],
                                ps[64 * half:64 * half + 64, JJ * hi:JJ * (hi + 1)],
                                bq_sb[64 * half:64 * half + 64, p:p + 1],
                            )

            wk_sb = []
            for ci in range(8):
                for p in range(2):
                    t = pp.tile([128, 128], bf16, tag=f"wk{ci}_{p}", name=f"wk{ci}_{p}")
                    eng = nc.sync if ci % 2 == 0 else nc.gpsimd
                    eng.dma_start(t[:], wkT_d[128 * ci:128 * (ci + 1), 128 * p:128 * (p + 1)])
                    wk_sb.append(t)
            wv_sb = []
            for ci in range(8):
                t = pp.tile([128, 256], bf16, tag=f"wv{ci}", name=f"wv{ci}")
                eng = nc.sync if ci % 2 == 0 else nc.gpsimd
                eng.dma_start(t[:], wvT_d[128 * ci:128 * (ci + 1), :])
                wv_sb.append(t)
            # Wo^T rows for this core's 256 feature channels (resident)
            wo_sb = []
            for j in range(2):
                t = pp.tile([128, D], bf16, tag=f"woH{j}", name=f"woH{j}")
                eng = nc.sync if j % 2 == 0 else nc.gpsimd
                eng.dma_start(t[:], woT_d[128 * j:128 * (j + 1), :])
                wo_sb.append(t)

            # K projection (transposed layout, 256 head-channels).
            # 8 chunks (nb, p) cycle through one B slot and one C slot.
            NPK = max(1, SCW // NQ)
            kps = {}
            for g in range((2 * NB + NPK - 1) // NPK):
                kps[g] = bigB(f"kps{g}")
            for idx in range(2 * NB):
                nb, p = divmod(idx, 2)
                ps_k = kps[idx // NPK][:, NQ * (idx % NPK):NQ * (idx % NPK + 1)]
                for ci in range(8):
                    kt_t = sp.tile([128, NQ], bf16, tag=f"kt{idx % 2}", bufs=4,
                                   name=f"kt{nb}_{p}_{ci}")
                    eng = nc.sync if ci % 2 == 0 else nc.gpsimd
                    eng.dma_start(kt_t[:], kT_d[128 * ci:128 * (ci + 1), NQ * nb:NQ * (nb + 1)])
                    nc.tensor.matmul(ps_k, wk_sb[2 * ci + p][:], kt_t[:],
                                     start=(ci == 0), stop=(ci == 7))
                for half in range(2):
                    hi = 2 * p + half
                    nc.vector.tensor_scalar_add(
                        KTs(hi)[:, NQ * nb:NQ * (nb + 1)],
                        ps_k[64 * half:64 * half + 64, :],
                        bk_sb[64 * half:64 * half + 64, p:p + 1],
                    )

            # V projection (natural layout) + bv via rank-1 matmul, into VA.
            # Each s-chunk psum (128, 256) sits in a 512-aligned slice so
            # concurrent slices never share a PSUM bank.
            NVG = min(4, max(1, S // 512))   # s-chunks per psum tile / vT load
            for grp in range(SCH // NVG):
                vps = bigC(f"vps{grp}")
                slices = [vps[:, 512 * i:512 * i + 256] for i in range(NVG)]
                for ci in range(8):
                    vtg = sp.tile([128, 128 * NVG], bf16, tag="vtg", bufs=4,
                                  name=f"vtg{grp}_{ci}")
                    eng = nc.sync if ci % 2 == 0 else nc.gpsimd
                    eng.dma_start(vtg[:], vT_d[128 * ci:128 * (ci + 1), 128 * NVG * grp:128 * NVG * (grp + 1)])
                    for i in range(NVG):
                        nc.tensor.matmul(slices[i], vtg[:, 128 * i:128 * (i + 1)], wv_sb[ci][:],
                                         start=(ci == 0), stop=False)
                for i in range(NVG):
                    sc = NVG * grp + i
                    nc.tensor.matmul(slices[i], onesrow_sb[:], bvrow_sb[:],
                                     start=False, stop=True)
                    nc.vector.tensor_copy(
                        VA[sc].rearrange("p (h x) -> p h x", h=4)[:, :, 0:64],
                        slices[i].rearrange("p (h d) -> p h d", h=4),
                    )

            # (1 - mask)^T resident in bf16 — loaded behind the projection DMAs
            maskb_sb = []
            for kc in range(KC):
                t = pp.tile([128, S], bf16, tag=f"mb{kc}", name=f"mb{kc}")
                eng = nc.sync if kc % 2 == 0 else nc.gpsimd
                eng.dma_start(t[:], maskb_d[128 * kc:128 * (kc + 1), :])
                maskb_sb.append(t)

            # ---------------- attention ----------------
            for hi in range(4):
                PVp = psC.tile([65, S], f32, tag="pv", name=f"pv{hi}")
                NH = S // SCW          # score pieces per kc
                NQH = SCW // NQ        # matmul N-chunks per piece
                for kc in range(KC):
                    for h2 in range(NH):
                        SC = bigB(f"sc{hi}_{kc}_{h2}")
                        for qb in range(NQH):
                            q0 = SCW * h2 + NQ * qb
                            nc.tensor.matmul(
                                SC[:, NQ * qb:NQ * (qb + 1)],
                                KTs(hi)[:, 128 * kc:128 * (kc + 1)],
                                QTs(hi)[:, q0:q0 + NQ],
                                start=True, stop=True,
                            )
                        E = sp.tile([128, SCW], bf16, tag="e", bufs=3 * NH,
                                    name=f"e{hi}_{kc}_{h2}")
                        nc.scalar.activation(E[:], SC[:], Exp, scale=1.0 / np.sqrt(DH))
                        Dt = sp.tile([128, SCW], bf16, tag="d", bufs=3 * NH,
                                     name=f"d{hi}_{kc}_{h2}")
                        nc.vector.tensor_mul(Dt[:], E[:], maskb_sb[kc][:, SCW * h2:SCW * (h2 + 1)])
                        for qb in range(NQH):
                            q0 = SCW * h2 + NQ * qb
                            nc.tensor.matmul(
                                PVp[:, q0:q0 + NQ],
                                VA[kc][:, 65 * hi:65 * hi + 65],
                                Dt[:, NQ * qb:NQ * (qb + 1)],
                                start=(kc == 0), stop=(kc == KC - 1),
                            )

                # evacuate P@V to SBUF so the PSUM slot frees for the next head
                PVs = sp.tile([65, S], f32, tag="pvs", bufs=1, name=f"pvs{hi}")
                nc.vector.tensor_copy(PVs[:], PVp[:])
                # normalize: R = 1/Z on partition 0, broadcast to 64 partitions
                # via a rank-1 PE outer product — no DRAM round trip
                rrow = sp.tile([1, S], f32r, tag="rrow", bufs=1, name=f"rrow{hi}")
                with nc.allow_low_precision(reason="1/Z rounded to f32r for the PE broadcast"):
                    nc.vector.reciprocal(rrow[:], PVs[64:65, :])
                for half in range(NH):
                    Rp = bigB(f"rb{hi}_{half}")
                    for qb in range(SCW // NQ):
                        nc.tensor.matmul(
                            Rp[0:64, NQ * qb:NQ * (qb + 1)],
                            onesrow_sb[:, 0:64],
                            rrow[:, SCW * half + NQ * qb:SCW * half + NQ * (qb + 1)],
                            start=True, stop=True,
                        )
                    nc.vector.tensor_mul(
                        featsb[hi // 2][64 * (hi % 2):64 * (hi % 2) + 64,
                                        SCW * half:SCW * (half + 1)],
                        PVs[0:64, SCW * half:SCW * (half + 1)],
                        Rp[0:64, :],
                    )

            # ---------------- output projection (partial, full width) ----------------
            # out_partial[D, S] = (Wo^T rows for local feats)^T @ feats; host
            # sums the 4 partials per batch and adds bo.
            for nb in range(NB):
                fb = [featsb[j][:, NQ * nb:NQ * (nb + 1)] for j in range(2)]
                slotB0 = bigB(f"psoB0_{nb}")
                slotB1 = bigB(f"psoB1_{nb}")
                slotC = bigC(f"psoC_{nb}")
                slots = [slotB0[:, 0:NQ], slotB0[:, NQ:2 * NQ],
                         slotB1[:, 0:NQ], slotB1[:, NQ:2 * NQ],
                         slotC[:, 0:NQ], slotC[:, NQ:2 * NQ],
                         slotC[:, 2 * NQ:3 * NQ], slotC[:, 3 * NQ:4 * NQ]]
                for p in range(8):
                    ps = slots[p]
                    for j in range(2):
                        nc.tensor.matmul(ps, wo_sb[j][:, 128 * p:128 * (p + 1)], fb[j],
                                         start=(j == 0), stop=(j == 1))
                    osb = sp.tile([128, NQ], f32, tag="osb", bufs=6, name=f"osb{nb}_{p}")
                    nc.scalar.activation(osb[:], ps, Ident)
                    eng = nc.sync if p % 2 == 0 else nc.gpsimd
                    eng.dma_start(out_d[128 * p:128 * (p + 1), NQ * nb:NQ * (nb + 1)], osb[:])

    nc.compile()
    return nc


def _get_module(S):
    if S not in _MODULES:
        _MODULES[S] = build_module(S)
    return _MODULES[S]


def host_shard(inputs, S):
    """Build the 8 per-core packed input maps from the full-size inputs."""
    OFF, NELEM = _layout(S)
    q = np.asarray(inputs["queries"], dtype=np.float32)
    k = np.asarray(inputs["keys"], dtype=np.float32)
    v = np.asarray(inputs["values"], dtype=np.float32)
    mask = np.asarray(inputs["mask"])
    Wq = np.asarray(inputs["Wq"], dtype=np.float32)
    Wk = np.asarray(inputs["Wk"], dtype=np.float32)
    Wv = np.asarray(inputs["Wv"], dtype=np.float32)
    Wo = np.asarray(inputs["Wo"], dtype=np.float32)
    bq = np.asarray(inputs["bq"], dtype=np.float32)
    bk = np.asarray(inputs["bk"], dtype=np.float32)
    bv = np.asarray(inputs["bv"], dtype=np.float32)

    JJ = S // 16
    maskb = (1 - mask[0, 0]).T.astype(ml_dtypes.bfloat16)
    wqT = Wq.T.astype(ml_dtypes.bfloat16)
    WkT = Wk.T.astype(ml_dtypes.bfloat16)
    WvT = Wv.T.astype(ml_dtypes.bfloat16)
    WoT = Wo.T.astype(ml_dtypes.bfloat16)
    kTs = [k[b].T.astype(ml_dtypes.bfloat16) for b in range(B)]
    vTs = [v[b].T.astype(ml_dtypes.bfloat16) for b in range(B)]
    bq_t = np.ascontiguousarray(bq.reshape(D // 128, 128).T)

    in_maps = []
    for c in range(NCORES):
        b, g = divmod(c, 4)
        heads = 4 * g + np.arange(4)
        # rows s = 16*jj + i, ordered [hi, jj]
        s_idx = (16 * np.arange(JJ)[None, :] + heads[:, None]).reshape(-1)
        ch = slice(256 * g, 256 * g + 256)

        pack = np.empty(NELEM, dtype=ml_dtypes.bfloat16)

        def put(name, arr):
            flat = np.ascontiguousarray(arr).ravel()
            pack[OFF[name]:OFF[name] + flat.size] = flat

        def putf(name, arr):
            flat = np.ascontiguousarray(arr, dtype=np.float32).ravel()
            pack[OFF[name]:OFF[name] + 2 * flat.size] = flat.view(ml_dtypes.bfloat16)

        put("qT", q[b][s_idx].T.astype(ml_dtypes.bfloat16))
        put("kT", kTs[b])
        put("vT", vTs[b])
        put("mb", maskb)
        put("wq", wqT)
        put("wk", WkT[:, ch])
        put("wv", WvT[:, ch])
        put("wo", WoT[ch, :])
        putf("bq", bq_t)
        putf("bk", bk[ch].reshape(2, 128).T)
        putf("bvrow", bv[ch].reshape(1, 256))
        putf("ones", np.ones((1, 128), np.float32))
        in_maps.append({"pack": pack})
    return in_maps


def assemble(results, S, bo):
    out = np.empty((B, S, D), dtype=np.float32)
    for b in range(B):
        acc = results[4 * b]["ofinal"].copy()
        for g in range(1, 4):
            acc += results[4 * b + g]["ofinal"]
        out[b] = acc.T + bo
    return out


def kernel(**inputs):
    S = int(np.asarray(inputs["queries"]).shape[1])
    nc = _get_module(S)
    in_maps = host_shard(inputs, S)
    res = run_bass_kernel_spmd(nc, in_maps, core_ids=list(range(NCORES)))
    bo = np.asarray(inputs["bo"], dtype=np.float32)
    return assemble(res.results, S, bo)
